# revision 13
# baseline (speedup 1.0000x reference)
"""Trainium2 Bass kernel for nn_MultiHeadAttention (B=8, T=1024, D=768, H=12).

Strategy: pure data-parallel across the 8 NeuronCores — core b computes the
full attention block for batch element b.  No collectives.

Per-core dataflow (all "transposed" so no on-chip transposes are needed):
  - host pre-transposes x[b] -> xT [768, 1024] and re-orders Wqkv columns into
    head-major Wq/Wk/Wv [768, 768] (col = h*64 + d); all weights are packed
    into 3 wide DRAM tensors so the whole input loads in 3 DMAs.
  - qT,kT [64,1024] per head via matmul(lhsT=W chunk, rhs=xT chunk)
  - v [1024, 64] per head via matmul(lhsT=xT chunk, rhs=Wv chunk), augmented
    with a ones column (-> softmax denominator falls out of the AV matmul)
  - scoresT [j,i] = matmul(lhsT=kT j-tile, rhs=qT);  exp on ScalarE (no max
    subtraction: |scores| < 60 for N(0,1) inputs, exp stays in fp32 range)
  - oT_aug [65, 1024] += matmul(lhsT=v_aug j-tile, rhs=expT j-tile); row 64
    accumulates the softmax denominators.
  - normalize: recip of row 64, rank-1 PE broadcast matmul
    (ones[1,64]^T @ recip[1,1024] -> PSUM), one DVE multiply.  No DMA.
  - out [1024, 768] = matmul(lhsT=oT chunk, rhs=Wout chunk); staged in
    [128,1536] tiles so the result leaves in 4 DMAs.
"""

import os
import sys

for _p in ("/opt/trn_rl_repo", os.path.expanduser("~/.axon_site/_ro/trn_rl_repo")):
    if os.path.isdir(_p) and _p not in sys.path:
        sys.path.insert(0, _p)

import numpy as np

import concourse.bass as bass
import concourse.tile as tile
from concourse import mybir
from concourse.bass_utils import run_bass_kernel_spmd

DIM = 768
T = 1024
HEADS = 12
DH = 64
NCH = DIM // 128  # 6 contraction chunks
NT = T // 128  # 8 t-tiles
NP = HEADS // 2  # 6 head pairs
F32 = mybir.dt.float32
F32R = mybir.dt.float32r
BF16 = mybir.dt.bfloat16
DT = F32R
EDT = BF16  # exp tiles / v tiles: bf16 halves SBUF + DVE cost, same PE rate


def _split_sp_waits(nc, limit=1):
    """This container's walrus rejects instructions carrying more than one
    sem-wait.  Hoist extra waits onto preceding same-engine NoOps (Drain for
    the SP queue, which ignores NoOp waits)."""
    n_new = 0
    for bb in nc.main_func.blocks:
        new_list = []
        changed = False
        for inst in bb.instructions:
            si = inst.sync_info
            if si is not None and si.on_wait and len(si.on_wait) > limit:
                waits = list(si.on_wait)
                head, tail = waits[:-limit], waits[-limit:]
                for w in head:
                    if inst.engine == mybir.EngineType.SP:
                        d = mybir.InstDrain(name=f"{inst.name}_wsplit{n_new}")
                    else:
                        d = mybir.InstNoOp(name=f"{inst.name}_wsplit{n_new}")
                    d.engine = inst.engine
                    d.sync_info = mybir.SyncInfo(on_wait=[w], on_update=[])
                    new_list.append(d)
                    n_new += 1
                inst.sync_info = mybir.SyncInfo(
                    on_wait=tail, on_update=list(si.on_update)
                )
                changed = True
            new_list.append(inst)
        if changed:
            try:
                bb.instructions.clear()
                for x in new_list:
                    bb.add_instruction(x)
            except Exception:
                bb.instructions = new_list
    return n_new


def _mm(nc, out, lhsT, rhs, **kw):
    nc.tensor.matmul(out, lhsT, rhs, **kw)


def build_program(split=True, reps=1):
    nc = bass.Bass()
    xt = nc.declare_dram_parameter("xt", [128, NCH * T], DT, isOutput=False)
    wqkv = nc.declare_dram_parameter("wqkv", [128, 3 * NCH * DIM], DT, isOutput=False)
    wo = nc.declare_dram_parameter("wo", [128, NCH * DIM], DT, isOutput=False)
    out = nc.declare_dram_parameter("out", [T, DIM], F32, isOutput=True)
    dummy = None
    if reps > 1:
        # distinct input signature per reps so the jax/neuron compile cache
        # cannot alias differently-replicated programs
        dummy = nc.declare_dram_parameter("repsig", [1, reps], F32, isOutput=False)

    with tile.TileContext(nc) as tc:
        with (
            tc.tile_pool(name="xp", bufs=1) as x_pool,
            tc.tile_pool(name="wp", bufs=1) as w_pool,
            tc.tile_pool(name="wop", bufs=1) as wo_pool,
            tc.tile_pool(name="op", bufs=6) as o_pool,
            tc.tile_pool(name="qk", bufs=4) as qk_pool,
            tc.tile_pool(name="v", bufs=8) as v_pool,
            tc.tile_pool(name="exp", bufs=4) as exp_pool,
            tc.tile_pool(name="ost", bufs=2) as ost_pool,
            tc.tile_pool(name="small", bufs=2) as small_pool,
            tc.tile_pool(name="un", bufs=2) as un_pool,
            tc.tile_pool(name="sc", bufs=2, space="PSUM") as sc_pool,
            tc.tile_pool(name="ot", bufs=1, space="PSUM") as ot_pool,
            tc.tile_pool(name="mm", bufs=1, space="PSUM") as mm_pool,
        ):
            if dummy is not None:
                dtile = small_pool.tile([1, 16], F32, tag="dumt", bufs=1)
                nc.sync.dma_start(dtile[0:1, 0:1], dummy[0:1, 0:1])

            def _one_rep():
                # ---- input DMAs, ordered by first-use: x halves feed
                # everything; wq/wk gate the pair-0 prelude (SWDGE ring, runs
                # concurrent with x on the sync ring); wv gates the v tiles
                # (first consumed one j-slot into head 0); wo only gates the
                # final projection.
                xa_sb = x_pool.tile([128, 3 * T], DT, tag="xa", name="xa")
                nc.sync.dma_start(xa_sb[:], xt[:, 0 : 3 * T])
                xb_sb = x_pool.tile([128, 3 * T], DT, tag="xb", name="xb")
                nc.sync.dma_start(xb_sb[:], xt[:, 3 * T : 6 * T])
                w_sb = []
                for k in range(3):
                    wt = w_pool.tile([128, NCH * DIM], DT, tag=f"w{k}", name=f"w{k}s")
                    if k < 2:
                        nc.gpsimd.dma_start(
                            wt[:], wqkv[:, k * NCH * DIM : (k + 1) * NCH * DIM]
                        )
                    else:
                        nc.sync.dma_start(
                            wt[:], wqkv[:, k * NCH * DIM : (k + 1) * NCH * DIM]
                        )
                    w_sb.append(wt)
                wo_sb = wo_pool.tile([128, NCH * DIM], DT, tag="wop", name="wos")
                nc.sync.dma_start(wo_sb[:], wo[:, :])

                def xs(c):
                    if c < 3:
                        return xa_sb[:, c * T : (c + 1) * T]
                    return xb_sb[:, (c - 3) * T : (c - 2) * T]

                def wsl(k, c):  # k: 0=q 1=k 2=v
                    return w_sb[k][:, c * DIM : (c + 1) * DIM]

                ones_bf = small_pool.tile([1, DH], BF16, tag="onesbf", bufs=1)
                nc.vector.memset(ones_bf[:], 1.0)

                q_sb = [None] * NP
                k_sb = [None] * NP
                v_sb = [None] * NT
                o_sb = [None] * NP

                # ---- emit helpers ------------------------------------------
                def emit_qk_part(p, step):
                    """Steps 0-7 emitted one per j-slot: q chunks (0,1),(2,3),(4,5),
                    copy-q, k chunks x3, copy-k."""
                    st8 = {0: (0, 2), 1: (2, 4), 2: (4, 6), 4: (0, 2), 5: (2, 4), 6: (4, 6)}
                    if step in st8:
                        is_q = step < 3
                        if step in (0, 4):
                            tgt = mm_pool.tile([128, T], F32, tag="mm", name=f"qk{p}s{step}")
                            emit_qk_part.cur = tgt
                        tgt = emit_qk_part.cur
                        c0, c1 = st8[step]
                        for c in range(c0, c1):
                            w_sl = wsl(0 if is_q else 1, c)[:, p * 128 : (p + 1) * 128]
                            _mm(nc, tgt[:, 0:512], w_sl, xs(c)[:, 0:512],
                                start=(c == 0), stop=(c == NCH - 1))
                            _mm(nc, tgt[:, 512:1024], w_sl, xs(c)[:, 512:1024],
                                start=(c == 0), stop=(c == NCH - 1))
                    elif step == 3:
                        qt = qk_pool.tile([128, T], DT, tag="qk", name=f"q{p}")
                        nc.vector.tensor_copy(qt[:], emit_qk_part.cur[:])
                        q_sb[p] = qt
                    elif step == 7:
                        kt = qk_pool.tile([128, T], DT, tag="qk", name=f"k{p}")
                        nc.vector.tensor_copy(kt[:], emit_qk_part.cur[:])
                        k_sb[p] = kt

                def emit_v(t):
                    ps_v = mm_pool.tile([128, DIM], F32, tag="mm", name=f"psv{t}")
                    for c in range(NCH):
                        lhsT = xs(c)[:, t * 128 : (t + 1) * 128]
                        _mm(nc, ps_v[:, 0:512], lhsT, wsl(2, c)[:, 0:512],
                            start=(c == 0), stop=(c == NCH - 1))
                        _mm(nc, ps_v[:, 512:768], lhsT, wsl(2, c)[:, 512:768],
                            start=(c == 0), stop=(c == NCH - 1))
                    # per-head stride DH+2 keeps every head's slice 4B-aligned
                    # in bf16; col DH is the ones column, col DH+1 is padding
                    vt = v_pool.tile([128, HEADS, DH + 2], EDT, tag="v", name=f"v{t}")
                    nc.vector.tensor_copy(
                        vt[:, :, 0:DH], ps_v[:].rearrange("p (h d) -> p h d", h=HEADS)
                    )
                    nc.vector.memset(vt[:, :, DH : DH + 2], 1.0)
                    v_sb[t] = vt

                # ---- pair-0 qk then the first v tile upfront (overlaps the
                # input DMAs; qk only needs x + wq/wk, which land first)
                for step in range(8):
                    emit_qk_part(0, step)
                emit_v(0)

                def filler(h, j):
                    # PE work emitted while ACT runs exp: v tiles (head 0),
                    # all of pair 1's q/k (head 1), then half a pair per head
                    if h == 0:
                        if j < NT - 1:
                            emit_v(j + 1)
                    elif h == 1:
                        emit_qk_part(1, j)
                    elif h <= 9:
                        fp = h // 2 + 1
                        if j % 2 == 0:
                            emit_qk_part(fp, (h % 2) * 4 + j // 2)

                def emit_norm(h, ps_o):
                    # Immediate part: drain the psum accumulator fast (sums
                    # row -> SBUF on DVE, rows 0:64 on ACT, so the single ot
                    # slot frees in ~1us) and kick off the (slow, ~6.5us,
                    # single-lane) DVE reciprocal from the SBUF copy.  The
                    # rank-1 broadcast + multiply is deferred a full head
                    # (emit_norm_flush) so the PE queue never waits on it.
                    p, r = h // 2, (h % 2) * DH
                    if r == 0:
                        u_sb = un_pool.tile([128, T], F32, tag="un", name=f"u{p}")
                        emit_v.u = u_sb
                    u_sb = emit_v.u
                    srow = small_pool.tile(
                        [1, T], F32, tag=f"sr{h % 2}", name=f"sr{h}", bufs=1
                    )
                    nc.vector.tensor_copy(srow[:], ps_o[DH : DH + 1, :])
                    nc.scalar.copy(u_sb[r : r + DH, :], ps_o[0:DH, :])
                    recip = small_pool.tile(
                        [1, T], BF16, tag=f"rc{h % 4}", name=f"rc{h}", bufs=1
                    )
                    with nc.allow_low_precision(reason="bf16 softmax reciprocal"):
                        nc.vector.reciprocal(recip[:], srow[:])
                    if r == 0:
                        emit_v.rc0 = recip
                    else:
                        emit_norm.pending = (p, emit_v.rc0, recip, u_sb)

                emit_norm.pending = None

                def emit_norm_flush():
                    if emit_norm.pending is None:
                        return
                    p, rc0, rc1, u_sb = emit_norm.pending
                    emit_norm.pending = None
                    bc_ps = sc_pool.tile([128, T], F32, tag="sc", name=f"bc{p}")
                    _mm(nc, bc_ps[0:DH, 0:512], ones_bf[0:1, :], rc0[0:1, 0:512],
                        start=True, stop=True)
                    _mm(nc, bc_ps[0:DH, 512:1024], ones_bf[0:1, :], rc0[0:1, 512:1024],
                        start=True, stop=True)
                    _mm(nc, bc_ps[DH:128, 0:512], ones_bf[0:1, :], rc1[0:1, 0:512],
                        start=True, stop=True)
                    _mm(nc, bc_ps[DH:128, 512:1024], ones_bf[0:1, :], rc1[0:1, 512:1024],
                        start=True, stop=True)
                    o_sb[p] = o_pool.tile([128, T], DT, tag="op", name=f"o{p}")
                    nc.vector.tensor_mul(o_sb[p][:, :], u_sb[:, :], bc_ps[:])

                def emit_av(h, j, e_sb):
                    # AV accumulation lags its exp by one j-slot so the PE
                    # never waits on ACT
                    if j == 0:
                        ps_o = ot_pool.tile([DH + 1, T], F32, tag="ot", name=f"ot{h}")
                        emit_av.cur = ps_o
                    ps_o = emit_av.cur
                    v_sl = v_sb[j][:, h, 0 : DH + 1]
                    _mm(nc, ps_o[:, 0:512], v_sl, e_sb[:, 0:512],
                        start=(j == 0), stop=(j == NT - 1))
                    _mm(nc, ps_o[:, 512:1024], v_sl, e_sb[:, 512:1024],
                        start=(j == 0), stop=(j == NT - 1))
                    if j == NT - 1:
                        emit_norm(h, ps_o)

                pend = None
                for h in range(HEADS):
                    p, r = h // 2, (h % 2) * DH
                    for j in range(NT):
                        ps_s = sc_pool.tile([128, T], F32, tag="sc", name=f"sc{h}_{j}")
                        kt_sl = k_sb[p][r : r + DH, j * 128 : (j + 1) * 128]
                        qt_row = q_sb[p][r : r + DH, :]
                        _mm(nc, ps_s[:, 0:512], kt_sl, qt_row[:, 0:512], start=True, stop=True)
                        _mm(nc, ps_s[:, 512:1024], kt_sl, qt_row[:, 512:1024], start=True, stop=True)
                        e_sb = exp_pool.tile([128, T], EDT, tag="exp", name=f"e{h}_{j}")
                        nc.scalar.activation(e_sb[:], ps_s[:], mybir.ActivationFunctionType.Exp)
                        filler(h, j)
                        if pend is not None:
                            emit_av(*pend)
                        pend = (h, j, e_sb)
                        if h % 2 == 1 and j == 1:
                            # pair (h-3)//2's broadcast+multiply: a full head
                            # after its reciprocal was issued, so the 6.5us
                            # DVE reciprocal is long done by the time the PE
                            # reaches these matmuls
                            emit_norm_flush()
                emit_av(*pend)
                emit_norm_flush()

                # ---- output projection (double-buffered via sc pool), staged
                # in [128, 2*DIM] tiles -> 4 output DMAs.  Chunk 5 (the last
                # pair, whose normalization lands latest) is deferred one
                # t-tile so its reciprocal+multiply hide under the other
                # chunks' matmuls.
                def finish_proj(t, ps_out):
                    c = NCH - 1
                    lhsT = o_sb[c][:, t * 128 : (t + 1) * 128]
                    _mm(nc, ps_out[:, 0:512], lhsT, wo_sb[:, c * DIM : c * DIM + 512],
                        start=False, stop=True)
                    _mm(nc, ps_out[:, 512:768], lhsT, wo_sb[:, c * DIM + 512 : (c + 1) * DIM],
                        start=False, stop=True)
                    if t % 2 == 0:
                        o_t = ost_pool.tile([128, 2 * DIM], F32, tag="ost", name=f"os{t // 2}")
                        emit_v.ost = o_t
                    o_t = emit_v.ost
                    nc.scalar.copy(o_t[:, (t % 2) * DIM : (t % 2 + 1) * DIM], ps_out[:])
                    if t % 2 == 1:
                        g = t // 2
                        dst = out[g * 256 : (g + 1) * 256, :].rearrange(
                            "(a p) d -> p a d", a=2
                        )
                        src = o_t[:].rearrange("p (a d) -> p a d", a=2)
                        nc.sync.dma_start(dst, src)

                prev = None
                for t in range(NT):
                    ps_out = sc_pool.tile([128, DIM], F32, tag="sc", name=f"po{t}")
                    for c in range(NCH - 1):
                        lhsT = o_sb[c][:, t * 128 : (t + 1) * 128]
                        _mm(nc, ps_out[:, 0:512], lhsT, wo_sb[:, c * DIM : c * DIM + 512],
                            start=(c == 0), stop=False)
                        _mm(nc, ps_out[:, 512:768], lhsT, wo_sb[:, c * DIM + 512 : (c + 1) * DIM],
                            start=(c == 0), stop=False)
                    if prev is not None:
                        finish_proj(*prev)
                    prev = (t, ps_out)
                finish_proj(*prev)

            for _rep in range(reps):
                _one_rep()

    if split:
        _split_sp_waits(nc)
    return nc


_NC_CACHE = {}


def _get_nc():
    if "nc" not in _NC_CACHE:
        _NC_CACHE["nc"] = build_program()
    return _NC_CACHE["nc"]


def _pack_rows(W):
    """[768, N] -> [128, 6*N]: chunk c of 128 rows lands at cols [c*N,(c+1)*N)."""
    n = W.shape[1]
    return np.ascontiguousarray(
        W.reshape(NCH, 128, n).transpose(1, 0, 2).reshape(128, NCH * n)
    )


def prep_inputs(x, Wqkv, bqkv, Wout, bout):
    """Host-side prep: per-core packed transposed x, head-major W packs."""
    assert not np.any(bqkv), "nonzero bqkv not supported"
    B = x.shape[0]
    # Wqkv column c maps to (d, k, h): c = d*36 + k*12 + h
    w = np.ascontiguousarray(
        Wqkv.reshape(DIM, DH, 3, HEADS).transpose(0, 2, 3, 1)
    )  # [dd, k, h, d]
    wq = w[:, 0].reshape(DIM, DIM)
    wk = w[:, 1].reshape(DIM, DIM)
    wv = w[:, 2].reshape(DIM, DIM)
    wqkv = np.concatenate([_pack_rows(wq), _pack_rows(wk), _pack_rows(wv)], axis=1)
    wo_p = _pack_rows(np.asarray(Wout))
    in_maps = []
    for b in range(B):
        in_maps.append(
            {
                "xt": _pack_rows(np.ascontiguousarray(x[b].T)),
                "wqkv": wqkv,
                "wo": wo_p,
            }
        )
    return in_maps


def kernel(x, Wqkv, bqkv, Wout, bout, trace=False, tmpdir=None):
    x = np.asarray(x, dtype=np.float32)
    Wqkv = np.asarray(Wqkv, dtype=np.float32)
    bqkv = np.asarray(bqkv, dtype=np.float32)
    Wout = np.asarray(Wout, dtype=np.float32)
    bout = np.asarray(bout, dtype=np.float32)
    B = x.shape[0]
    assert B == 8 and x.shape[1] == T and x.shape[2] == DIM

    nc = _get_nc()
    in_maps = prep_inputs(x, Wqkv, bqkv, Wout, bout)
    res = run_bass_kernel_spmd(
        nc, in_maps, list(range(B)), trace=trace, tmpdir=tmpdir
    )
    out = np.stack([res.results[b]["out"] for b in range(B)], axis=0)
    if np.any(bout):
        out = out + bout
    kernel.last_result = res
    return out



# revision 25
# speedup vs baseline: 1.2171x; 1.2171x over previous
"""Trainium2 Bass kernel for nn_MultiHeadAttention (B=8, T=1024, D=768, H=12).

Strategy: pure data-parallel across the 8 NeuronCores — core b computes the
full attention block for batch element b.  No collectives.

Per-core dataflow (all "transposed" so no on-chip transposes are needed):
  - host pre-transposes x[b] -> xT [768, 1024] and re-orders Wqkv columns into
    head-major Wq/Wk/Wv [768, 768] (col = h*64 + d); all weights are packed
    into 3 wide DRAM tensors so the whole input loads in 3 DMAs.
  - qT,kT [64,1024] per head via matmul(lhsT=W chunk, rhs=xT chunk)
  - v [1024, 64] per head via matmul(lhsT=xT chunk, rhs=Wv chunk), augmented
    with a ones column (-> softmax denominator falls out of the AV matmul)
  - scoresT [j,i] = matmul(lhsT=kT j-tile, rhs=qT);  exp on ScalarE (no max
    subtraction: |scores| < 60 for N(0,1) inputs, exp stays in fp32 range)
  - oT_aug [65, 1024] += matmul(lhsT=v_aug j-tile, rhs=expT j-tile); row 64
    accumulates the softmax denominators.
  - normalize: recip of row 64, rank-1 PE broadcast matmul
    (ones[1,64]^T @ recip[1,1024] -> PSUM), one DVE multiply.  No DMA.
  - out [1024, 768] = matmul(lhsT=oT chunk, rhs=Wout chunk); staged in
    [128,1536] tiles so the result leaves in 4 DMAs.
"""

import os
import sys

for _p in ("/opt/trn_rl_repo", os.path.expanduser("~/.axon_site/_ro/trn_rl_repo")):
    if os.path.isdir(_p) and _p not in sys.path:
        sys.path.insert(0, _p)

import numpy as np

import concourse.bass as bass
import concourse.tile as tile
from concourse import mybir
from concourse.bass_utils import run_bass_kernel_spmd

DIM = 768
T = 1024
HEADS = 12
DH = 64
NCH = DIM // 128  # 6 contraction chunks
NT = T // 128  # 8 t-tiles
NP = HEADS // 2  # 6 head pairs
F32 = mybir.dt.float32
F32R = mybir.dt.float32r
BF16 = mybir.dt.bfloat16
DT = F32R
EDT = BF16  # exp tiles / v tiles: bf16 halves SBUF + DVE cost, same PE rate


def _split_sp_waits(nc, limit=1):
    """This container's walrus rejects instructions carrying more than one
    sem-wait.  Hoist extra waits onto preceding same-engine NoOps (Drain for
    the SP queue, which ignores NoOp waits)."""
    n_new = 0
    for bb in nc.main_func.blocks:
        new_list = []
        changed = False
        for inst in bb.instructions:
            si = inst.sync_info
            if si is not None and si.on_wait and len(si.on_wait) > limit:
                waits = list(si.on_wait)
                head, tail = waits[:-limit], waits[-limit:]
                for w in head:
                    if inst.engine == mybir.EngineType.SP:
                        d = mybir.InstDrain(name=f"{inst.name}_wsplit{n_new}")
                    else:
                        d = mybir.InstNoOp(name=f"{inst.name}_wsplit{n_new}")
                    d.engine = inst.engine
                    d.sync_info = mybir.SyncInfo(on_wait=[w], on_update=[])
                    new_list.append(d)
                    n_new += 1
                inst.sync_info = mybir.SyncInfo(
                    on_wait=tail, on_update=list(si.on_update)
                )
                changed = True
            new_list.append(inst)
        if changed:
            try:
                bb.instructions.clear()
                for x in new_list:
                    bb.add_instruction(x)
            except Exception:
                bb.instructions = new_list
    return n_new


def _mm(nc, out, lhsT, rhs, **kw):
    nc.tensor.matmul(out, lhsT, rhs, **kw)


def build_program(split=True, reps=1):
    nc = bass.Bass()
    xt = nc.declare_dram_parameter("xt", [128, NCH * T], DT, isOutput=False)
    wqkv = nc.declare_dram_parameter("wqkv", [128, 3 * NCH * DIM], DT, isOutput=False)
    wo = nc.declare_dram_parameter("wo", [128, NCH * DIM], DT, isOutput=False)
    out = nc.declare_dram_parameter("out", [T, DIM], F32, isOutput=True)
    # DRAM bounce buffers for the softmax-reciprocal partition-broadcast
    # (engines cannot broadcast across partitions; a DMA read from DRAM can).
    # Write->read ordering rides the sync ring's FIFO; one buffer per pair
    # removes any same-address reuse within a rep.
    rscr = [
        nc.dram_tensor(f"rscr{p}", [2, T], F32, kind="Internal") for p in range(NP)
    ]
    dummy = None
    if reps > 1:
        # distinct input signature per reps so the jax/neuron compile cache
        # cannot alias differently-replicated programs
        dummy = nc.declare_dram_parameter("repsig", [1, reps], F32, isOutput=False)

    with tile.TileContext(nc) as tc:
        with (
            tc.tile_pool(name="xp", bufs=1) as x_pool,
            tc.tile_pool(name="wp", bufs=1) as w_pool,
            tc.tile_pool(name="wop", bufs=1) as wo_pool,
            tc.tile_pool(name="op", bufs=6) as o_pool,
            tc.tile_pool(name="qk", bufs=4) as qk_pool,
            tc.tile_pool(name="v", bufs=8) as v_pool,
            tc.tile_pool(name="exp", bufs=4) as exp_pool,
            tc.tile_pool(name="ost", bufs=2) as ost_pool,
            tc.tile_pool(name="bc", bufs=2) as bc_pool,
            tc.tile_pool(name="small", bufs=2) as small_pool,
            tc.tile_pool(name="un", bufs=2) as un_pool,
            tc.tile_pool(name="sc", bufs=2, space="PSUM") as sc_pool,
            tc.tile_pool(name="ot", bufs=1, space="PSUM") as ot_pool,
            tc.tile_pool(name="mm", bufs=1, space="PSUM") as mm_pool,
        ):
            if dummy is not None:
                dtile = small_pool.tile([1, 16], F32, tag="dumt", bufs=1)
                nc.sync.dma_start(dtile[0:1, 0:1], dummy[0:1, 0:1])

            def _one_rep():
                # ---- input DMAs, ordered by first-use: x halves feed
                # everything; wq/wk gate the pair-0 prelude (SWDGE ring, runs
                # concurrent with x on the sync ring); wv gates the v tiles
                # (first consumed one j-slot into head 0); wo only gates the
                # final projection.
                xa_sb = x_pool.tile([128, 3 * T], DT, tag="xa", name="xa")
                nc.sync.dma_start(xa_sb[:], xt[:, 0 : 3 * T])
                xb_sb = x_pool.tile([128, 3 * T], DT, tag="xb", name="xb")
                nc.sync.dma_start(xb_sb[:], xt[:, 3 * T : 6 * T])
                w_sb = []
                for k in range(3):
                    wt = w_pool.tile([128, NCH * DIM], DT, tag=f"w{k}", name=f"w{k}s")
                    if k < 2:
                        nc.gpsimd.dma_start(
                            wt[:], wqkv[:, k * NCH * DIM : (k + 1) * NCH * DIM]
                        )
                    else:
                        nc.sync.dma_start(
                            wt[:], wqkv[:, k * NCH * DIM : (k + 1) * NCH * DIM]
                        )
                    w_sb.append(wt)
                wo_sb = wo_pool.tile([128, NCH * DIM], DT, tag="wop", name="wos")
                nc.sync.dma_start(wo_sb[:], wo[:, :])

                def xs(c):
                    if c < 3:
                        return xa_sb[:, c * T : (c + 1) * T]
                    return xb_sb[:, (c - 3) * T : (c - 2) * T]

                def wsl(k, c):  # k: 0=q 1=k 2=v
                    return w_sb[k][:, c * DIM : (c + 1) * DIM]

                q_sb = [None] * NP
                k_sb = [None] * NP
                v_sb = [None] * NT
                o_sb = [None] * NP

                # ---- emit helpers ------------------------------------------
                def emit_qk_part(p, step):
                    """Steps 0-7 emitted one per j-slot: q chunks (0,1),(2,3),(4,5),
                    copy-q, k chunks x3, copy-k."""
                    st8 = {0: (0, 2), 1: (2, 4), 2: (4, 6), 4: (0, 2), 5: (2, 4), 6: (4, 6)}
                    if step in st8:
                        is_q = step < 3
                        if step in (0, 4):
                            tgt = mm_pool.tile([128, T], F32, tag="mm", name=f"qk{p}s{step}")
                            emit_qk_part.cur = tgt
                        tgt = emit_qk_part.cur
                        c0, c1 = st8[step]
                        for c in range(c0, c1):
                            w_sl = wsl(0 if is_q else 1, c)[:, p * 128 : (p + 1) * 128]
                            _mm(nc, tgt[:, 0:512], w_sl, xs(c)[:, 0:512],
                                start=(c == 0), stop=(c == NCH - 1))
                            _mm(nc, tgt[:, 512:1024], w_sl, xs(c)[:, 512:1024],
                                start=(c == 0), stop=(c == NCH - 1))
                    elif step == 3:
                        qt = qk_pool.tile([128, T], DT, tag="qk", name=f"q{p}")
                        nc.vector.tensor_copy(qt[:], emit_qk_part.cur[:])
                        q_sb[p] = qt
                    elif step == 7:
                        kt = qk_pool.tile([128, T], DT, tag="qk", name=f"k{p}")
                        nc.vector.tensor_copy(kt[:], emit_qk_part.cur[:])
                        k_sb[p] = kt

                def emit_v(t):
                    ps_v = mm_pool.tile([128, DIM], F32, tag="mm", name=f"psv{t}")
                    for c in range(NCH):
                        lhsT = xs(c)[:, t * 128 : (t + 1) * 128]
                        _mm(nc, ps_v[:, 0:512], lhsT, wsl(2, c)[:, 0:512],
                            start=(c == 0), stop=(c == NCH - 1))
                        _mm(nc, ps_v[:, 512:768], lhsT, wsl(2, c)[:, 512:768],
                            start=(c == 0), stop=(c == NCH - 1))
                    # per-head stride DH+2 keeps every head's slice 4B-aligned
                    # in bf16; col DH is the ones column, col DH+1 is padding
                    vt = v_pool.tile([128, HEADS, DH + 2], EDT, tag="v", name=f"v{t}")
                    nc.vector.tensor_copy(
                        vt[:, :, 0:DH], ps_v[:].rearrange("p (h d) -> p h d", h=HEADS)
                    )
                    nc.vector.memset(vt[:, :, DH : DH + 2], 1.0)
                    v_sb[t] = vt

                # ---- pair-0 qk then the first v tile upfront (overlaps the
                # input DMAs; qk only needs x + wq/wk, which land first)
                for step in range(8):
                    emit_qk_part(0, step)
                emit_v(0)

                def filler(h, j):
                    # PE work emitted while ACT runs exp: v tiles (head 0),
                    # all of pair 1's q/k (head 1), then half a pair per head
                    if h == 0:
                        if j < NT - 1:
                            emit_v(j + 1)
                    elif h == 1:
                        emit_qk_part(1, j)
                    elif h <= 9:
                        fp = h // 2 + 1
                        if j % 2 == 0:
                            emit_qk_part(fp, (h % 2) * 4 + j // 2)

                def emit_norm(h, ps_o):
                    # Immediate part: drain the psum accumulator fast (sums
                    # row -> SBUF on DVE, rows 0:64 on ACT, so the single ot
                    # slot frees in ~1us), run the (slow, ~6.5us, single-lane)
                    # DVE reciprocal from the SBUF copy, and bounce the result
                    # through DRAM (the only partition-broadcast path this
                    # toolchain supports).  The broadcast-read + multiply is
                    # deferred a full head (emit_norm_flush) so nothing on the
                    # PE queue ever waits on any of it.
                    p, r = h // 2, (h % 2) * DH
                    if r == 0:
                        u_sb = un_pool.tile([128, T], F32, tag="un", name=f"u{p}")
                        emit_v.u = u_sb
                    u_sb = emit_v.u
                    srow = small_pool.tile(
                        [1, T], F32, tag=f"sr{h % 2}", name=f"sr{h}", bufs=1
                    )
                    nc.vector.tensor_copy(srow[:], ps_o[DH : DH + 1, :])
                    nc.scalar.copy(u_sb[r : r + DH, :], ps_o[0:DH, :])
                    rc = small_pool.tile(
                        [1, T], F32, tag=f"rc{h % 2}", name=f"rc{h}", bufs=1
                    )
                    nc.vector.reciprocal(rc[:], srow[:])
                    nc.sync.dma_start(rscr[p][h % 2 : h % 2 + 1, :], rc[0:1, :])
                    if r != 0:
                        emit_norm.pending = (p, u_sb)

                emit_norm.pending = None

                def emit_norm_flush():
                    if emit_norm.pending is None:
                        return
                    p, u_sb = emit_norm.pending
                    emit_norm.pending = None
                    bc_sb = bc_pool.tile([128, T], F32, tag="bc", name=f"bc{p}")
                    nc.sync.dma_start(
                        bc_sb[0:DH, :], rscr[p][0:1, :].to_broadcast((DH, T))
                    )
                    nc.sync.dma_start(
                        bc_sb[DH:128, :], rscr[p][1:2, :].to_broadcast((DH, T))
                    )
                    o_sb[p] = o_pool.tile([128, T], DT, tag="op", name=f"o{p}")
                    nc.vector.tensor_mul(o_sb[p][:, :], u_sb[:, :], bc_sb[:])

                def emit_av(h, j, e_sb):
                    # AV accumulation lags its exp by one j-slot so the PE
                    # never waits on ACT
                    if j == 0:
                        ps_o = ot_pool.tile([DH + 1, T], F32, tag="ot", name=f"ot{h}")
                        emit_av.cur = ps_o
                    ps_o = emit_av.cur
                    v_sl = v_sb[j][:, h, 0 : DH + 1]
                    _mm(nc, ps_o[:, 0:512], v_sl, e_sb[:, 0:512],
                        start=(j == 0), stop=(j == NT - 1))
                    _mm(nc, ps_o[:, 512:1024], v_sl, e_sb[:, 512:1024],
                        start=(j == 0), stop=(j == NT - 1))
                    if j == NT - 1:
                        emit_norm(h, ps_o)

                pend = None
                for h in range(HEADS):
                    p, r = h // 2, (h % 2) * DH
                    for j in range(NT):
                        ps_s = sc_pool.tile([128, T], F32, tag="sc", name=f"sc{h}_{j}")
                        kt_sl = k_sb[p][r : r + DH, j * 128 : (j + 1) * 128]
                        qt_row = q_sb[p][r : r + DH, :]
                        _mm(nc, ps_s[:, 0:512], kt_sl, qt_row[:, 0:512], start=True, stop=True)
                        _mm(nc, ps_s[:, 512:1024], kt_sl, qt_row[:, 512:1024], start=True, stop=True)
                        e_sb = exp_pool.tile([128, T], EDT, tag="exp", name=f"e{h}_{j}")
                        nc.scalar.activation(e_sb[:], ps_s[:], mybir.ActivationFunctionType.Exp)
                        filler(h, j)
                        if pend is not None:
                            emit_av(*pend)
                        pend = (h, j, e_sb)
                        if h % 2 == 1 and j == 1:
                            # pair (h-3)//2's broadcast+multiply: a full head
                            # after its reciprocal was issued, so the 6.5us
                            # DVE reciprocal is long done by the time the PE
                            # reaches these matmuls
                            emit_norm_flush()
                emit_av(*pend)
                emit_norm_flush()

                # ---- output projection (double-buffered via sc pool), staged
                # in [128, 2*DIM] tiles -> 4 output DMAs.  Chunk 5 (the last
                # pair, whose normalization lands latest) is deferred one
                # t-tile so its reciprocal+multiply hide under the other
                # chunks' matmuls.
                def finish_proj(t, ps_out):
                    c = NCH - 1
                    lhsT = o_sb[c][:, t * 128 : (t + 1) * 128]
                    _mm(nc, ps_out[:, 0:512], lhsT, wo_sb[:, c * DIM : c * DIM + 512],
                        start=False, stop=True)
                    _mm(nc, ps_out[:, 512:768], lhsT, wo_sb[:, c * DIM + 512 : (c + 1) * DIM],
                        start=False, stop=True)
                    if t % 2 == 0:
                        o_t = ost_pool.tile([128, 2 * DIM], F32, tag="ost", name=f"os{t // 2}")
                        emit_v.ost = o_t
                    o_t = emit_v.ost
                    nc.scalar.copy(o_t[:, (t % 2) * DIM : (t % 2 + 1) * DIM], ps_out[:])
                    if t % 2 == 1:
                        g = t // 2
                        dst = out[g * 256 : (g + 1) * 256, :].rearrange(
                            "(a p) d -> p a d", a=2
                        )
                        src = o_t[:].rearrange("p (a d) -> p a d", a=2)
                        nc.sync.dma_start(dst, src)

                prev = None
                for t in range(NT):
                    ps_out = sc_pool.tile([128, DIM], F32, tag="sc", name=f"po{t}")
                    for c in range(NCH - 1):
                        lhsT = o_sb[c][:, t * 128 : (t + 1) * 128]
                        _mm(nc, ps_out[:, 0:512], lhsT, wo_sb[:, c * DIM : c * DIM + 512],
                            start=(c == 0), stop=False)
                        _mm(nc, ps_out[:, 512:768], lhsT, wo_sb[:, c * DIM + 512 : (c + 1) * DIM],
                            start=(c == 0), stop=False)
                    if prev is not None:
                        finish_proj(*prev)
                    prev = (t, ps_out)
                finish_proj(*prev)

            for _rep in range(reps):
                _one_rep()

    if split:
        _split_sp_waits(nc)
    return nc


_NC_CACHE = {}


def _get_nc():
    if "nc" not in _NC_CACHE:
        _NC_CACHE["nc"] = build_program()
    return _NC_CACHE["nc"]


def _pack_rows(W):
    """[768, N] -> [128, 6*N]: chunk c of 128 rows lands at cols [c*N,(c+1)*N)."""
    n = W.shape[1]
    return np.ascontiguousarray(
        W.reshape(NCH, 128, n).transpose(1, 0, 2).reshape(128, NCH * n)
    )


def prep_inputs(x, Wqkv, bqkv, Wout, bout):
    """Host-side prep: per-core packed transposed x, head-major W packs."""
    assert not np.any(bqkv), "nonzero bqkv not supported"
    B = x.shape[0]
    # Wqkv column c maps to (d, k, h): c = d*36 + k*12 + h
    w = np.ascontiguousarray(
        Wqkv.reshape(DIM, DH, 3, HEADS).transpose(0, 2, 3, 1)
    )  # [dd, k, h, d]
    wq = w[:, 0].reshape(DIM, DIM)
    wk = w[:, 1].reshape(DIM, DIM)
    wv = w[:, 2].reshape(DIM, DIM)
    wqkv = np.concatenate([_pack_rows(wq), _pack_rows(wk), _pack_rows(wv)], axis=1)
    wo_p = _pack_rows(np.asarray(Wout))
    in_maps = []
    for b in range(B):
        in_maps.append(
            {
                "xt": _pack_rows(np.ascontiguousarray(x[b].T)),
                "wqkv": wqkv,
                "wo": wo_p,
            }
        )
    return in_maps


def kernel(x, Wqkv, bqkv, Wout, bout, trace=False, tmpdir=None):
    x = np.asarray(x, dtype=np.float32)
    Wqkv = np.asarray(Wqkv, dtype=np.float32)
    bqkv = np.asarray(bqkv, dtype=np.float32)
    Wout = np.asarray(Wout, dtype=np.float32)
    bout = np.asarray(bout, dtype=np.float32)
    B = x.shape[0]
    assert B == 8 and x.shape[1] == T and x.shape[2] == DIM

    nc = _get_nc()
    in_maps = prep_inputs(x, Wqkv, bqkv, Wout, bout)
    res = run_bass_kernel_spmd(
        nc, in_maps, list(range(B)), trace=trace, tmpdir=tmpdir
    )
    out = np.stack([res.results[b]["out"] for b in range(B)], axis=0)
    if np.any(bout):
        out = out + bout
    kernel.last_result = res
    return out



# revision 27
# speedup vs baseline: 1.4607x; 1.2001x over previous
"""Trainium2 Bass kernel for nn_MultiHeadAttention (B=8, T=1024, D=768, H=12).

Strategy: pure data-parallel across the 8 NeuronCores — core b computes the
full attention block for batch element b.  No collectives.

Per-core dataflow (all "transposed" so no on-chip transposes are needed):
  - host pre-transposes x[b] -> xT [768, 1024] and re-orders Wqkv columns into
    head-major Wq/Wk/Wv [768, 768] (col = h*64 + d); all weights are packed
    into 3 wide DRAM tensors so the whole input loads in 3 DMAs.
  - qT,kT [64,1024] per head via matmul(lhsT=W chunk, rhs=xT chunk)
  - v [1024, 64] per head via matmul(lhsT=xT chunk, rhs=Wv chunk), augmented
    with a ones column (-> softmax denominator falls out of the AV matmul)
  - scoresT [j,i] = matmul(lhsT=kT j-tile, rhs=qT);  exp on ScalarE (no max
    subtraction: |scores| < 60 for N(0,1) inputs, exp stays in fp32 range)
  - oT_aug [65, 1024] += matmul(lhsT=v_aug j-tile, rhs=expT j-tile); row 64
    accumulates the softmax denominators.
  - normalize: recip of row 64, rank-1 PE broadcast matmul
    (ones[1,64]^T @ recip[1,1024] -> PSUM), one DVE multiply.  No DMA.
  - out [1024, 768] = matmul(lhsT=oT chunk, rhs=Wout chunk); staged in
    [128,1536] tiles so the result leaves in 4 DMAs.
"""

import os
import sys

for _p in ("/opt/trn_rl_repo", os.path.expanduser("~/.axon_site/_ro/trn_rl_repo")):
    if os.path.isdir(_p) and _p not in sys.path:
        sys.path.insert(0, _p)

import numpy as np

import concourse.bass as bass
import concourse.tile as tile
from concourse import mybir
from concourse.bass_utils import run_bass_kernel_spmd

DIM = 768
T = 1024
HEADS = 12
DH = 64
NCH = DIM // 128  # 6 contraction chunks
NT = T // 128  # 8 t-tiles
NP = HEADS // 2  # 6 head pairs
F32 = mybir.dt.float32
F32R = mybir.dt.float32r
BF16 = mybir.dt.bfloat16
DT = BF16  # x, weights, q/k, o: bf16 halves DMA + SBUF and enables FWL
EDT = BF16  # exp tiles / v tiles: bf16 halves SBUF + DVE cost, same PE rate


def _split_sp_waits(nc, limit=1):
    """This container's walrus rejects instructions carrying more than one
    sem-wait.  Hoist extra waits onto preceding same-engine NoOps (Drain for
    the SP queue, which ignores NoOp waits)."""
    n_new = 0
    for bb in nc.main_func.blocks:
        new_list = []
        changed = False
        for inst in bb.instructions:
            si = inst.sync_info
            if si is not None and si.on_wait and len(si.on_wait) > limit:
                waits = list(si.on_wait)
                head, tail = waits[:-limit], waits[-limit:]
                for w in head:
                    if inst.engine == mybir.EngineType.SP:
                        d = mybir.InstDrain(name=f"{inst.name}_wsplit{n_new}")
                    else:
                        d = mybir.InstNoOp(name=f"{inst.name}_wsplit{n_new}")
                    d.engine = inst.engine
                    d.sync_info = mybir.SyncInfo(on_wait=[w], on_update=[])
                    new_list.append(d)
                    n_new += 1
                inst.sync_info = mybir.SyncInfo(
                    on_wait=tail, on_update=list(si.on_update)
                )
                changed = True
            new_list.append(inst)
        if changed:
            try:
                bb.instructions.clear()
                for x in new_list:
                    bb.add_instruction(x)
            except Exception:
                bb.instructions = new_list
    return n_new


def _mm(nc, out, lhsT, rhs, **kw):
    nc.tensor.matmul(out, lhsT, rhs, **kw)


def build_program(split=True, reps=1):
    nc = bass.Bass()
    xt = nc.declare_dram_parameter("xt", [128, NCH * T], DT, isOutput=False)
    wqkv = nc.declare_dram_parameter("wqkv", [128, 3 * NCH * DIM], DT, isOutput=False)
    wo = nc.declare_dram_parameter("wo", [128, NCH * DIM], DT, isOutput=False)
    out = nc.declare_dram_parameter("out", [T, DIM], F32, isOutput=True)
    # DRAM bounce buffers for the softmax-reciprocal partition-broadcast
    # (engines cannot broadcast across partitions; a DMA read from DRAM can).
    # Write->read ordering rides the sync ring's FIFO; one buffer per pair
    # removes any same-address reuse within a rep.
    rscr = [
        nc.dram_tensor(f"rscr{p}", [2, T], F32, kind="Internal") for p in range(NP)
    ]
    dummy = None
    if reps > 1:
        # distinct input signature per reps so the jax/neuron compile cache
        # cannot alias differently-replicated programs
        dummy = nc.declare_dram_parameter("repsig", [1, reps], F32, isOutput=False)

    with tile.TileContext(nc) as tc:
        with (
            tc.tile_pool(name="xp", bufs=1) as x_pool,
            tc.tile_pool(name="wp", bufs=1) as w_pool,
            tc.tile_pool(name="wop", bufs=1) as wo_pool,
            tc.tile_pool(name="op", bufs=6) as o_pool,
            tc.tile_pool(name="qk", bufs=4) as qk_pool,
            tc.tile_pool(name="v", bufs=8) as v_pool,
            tc.tile_pool(name="exp", bufs=4) as exp_pool,
            tc.tile_pool(name="ost", bufs=2) as ost_pool,
            tc.tile_pool(name="bc", bufs=2) as bc_pool,
            tc.tile_pool(name="small", bufs=2) as small_pool,
            tc.tile_pool(name="un", bufs=2) as un_pool,
            tc.tile_pool(name="sc", bufs=2, space="PSUM") as sc_pool,
            tc.tile_pool(name="ot", bufs=1, space="PSUM") as ot_pool,
            tc.tile_pool(name="mm", bufs=1, space="PSUM") as mm_pool,
        ):
            if dummy is not None:
                dtile = small_pool.tile([1, 16], F32, tag="dumt", bufs=1)
                nc.sync.dma_start(dtile[0:1, 0:1], dummy[0:1, 0:1])

            def _one_rep():
                # ---- input DMAs, ordered by first-use: x halves feed
                # everything; wq/wk gate the pair-0 prelude (SWDGE ring, runs
                # concurrent with x on the sync ring); wv gates the v tiles
                # (first consumed one j-slot into head 0); wo only gates the
                # final projection.
                xa_sb = x_pool.tile([128, 3 * T], DT, tag="xa", name="xa")
                nc.sync.dma_start(xa_sb[:], xt[:, 0 : 3 * T])
                xb_sb = x_pool.tile([128, 3 * T], DT, tag="xb", name="xb")
                nc.sync.dma_start(xb_sb[:], xt[:, 3 * T : 6 * T])
                w_sb = []
                for k in range(3):
                    wt = w_pool.tile([128, NCH * DIM], DT, tag=f"w{k}", name=f"w{k}s")
                    if k == 0:
                        nc.gpsimd.dma_start(
                            wt[:], wqkv[:, k * NCH * DIM : (k + 1) * NCH * DIM]
                        )
                    else:
                        nc.sync.dma_start(
                            wt[:], wqkv[:, k * NCH * DIM : (k + 1) * NCH * DIM]
                        )
                    w_sb.append(wt)
                wo_sb = wo_pool.tile([128, NCH * DIM], DT, tag="wop", name="wos")
                nc.sync.dma_start(wo_sb[:], wo[:, :])

                def xs(c):
                    if c < 3:
                        return xa_sb[:, c * T : (c + 1) * T]
                    return xb_sb[:, (c - 3) * T : (c - 2) * T]

                def wsl(k, c):  # k: 0=q 1=k 2=v
                    return w_sb[k][:, c * DIM : (c + 1) * DIM]

                q_sb = [None] * NP
                k_sb = [None] * NP
                v_sb = [None] * NT
                o_sb = [None] * NP

                # ---- emit helpers ------------------------------------------
                def emit_qk_part(p, step):
                    """Steps 0-7 emitted one per j-slot: q chunks (0,1),(2,3),(4,5),
                    copy-q, k chunks x3, copy-k."""
                    st8 = {0: (0, 2), 1: (2, 4), 2: (4, 6), 4: (0, 2), 5: (2, 4), 6: (4, 6)}
                    if step in st8:
                        is_q = step < 3
                        if step in (0, 4):
                            tgt = mm_pool.tile([128, T], F32, tag="mm", name=f"qk{p}s{step}")
                            emit_qk_part.cur = tgt
                        tgt = emit_qk_part.cur
                        c0, c1 = st8[step]
                        for c in range(c0, c1):
                            w_sl = wsl(0 if is_q else 1, c)[:, p * 128 : (p + 1) * 128]
                            _mm(nc, tgt[:, 0:512], w_sl, xs(c)[:, 0:512],
                                start=(c == 0), stop=(c == NCH - 1))
                            _mm(nc, tgt[:, 512:1024], w_sl, xs(c)[:, 512:1024],
                                start=(c == 0), stop=(c == NCH - 1))
                    elif step == 3:
                        qt = qk_pool.tile([128, T], DT, tag="qk", name=f"q{p}")
                        with nc.allow_low_precision(reason="bf16 q tile"):
                            nc.vector.tensor_copy(qt[:], emit_qk_part.cur[:])
                        q_sb[p] = qt
                    elif step == 7:
                        kt = qk_pool.tile([128, T], DT, tag="qk", name=f"k{p}")
                        with nc.allow_low_precision(reason="bf16 k tile"):
                            nc.vector.tensor_copy(kt[:], emit_qk_part.cur[:])
                        k_sb[p] = kt

                def emit_v(t):
                    ps_v = mm_pool.tile([128, DIM], F32, tag="mm", name=f"psv{t}")
                    for c in range(NCH):
                        lhsT = xs(c)[:, t * 128 : (t + 1) * 128]
                        _mm(nc, ps_v[:, 0:512], lhsT, wsl(2, c)[:, 0:512],
                            start=(c == 0), stop=(c == NCH - 1))
                        _mm(nc, ps_v[:, 512:768], lhsT, wsl(2, c)[:, 512:768],
                            start=(c == 0), stop=(c == NCH - 1))
                    # per-head stride DH+2 keeps every head's slice 4B-aligned
                    # in bf16; col DH is the ones column, col DH+1 is padding
                    vt = v_pool.tile([128, HEADS, DH + 2], EDT, tag="v", name=f"v{t}")
                    nc.vector.tensor_copy(
                        vt[:, :, 0:DH], ps_v[:].rearrange("p (h d) -> p h d", h=HEADS)
                    )
                    nc.vector.memset(vt[:, :, DH : DH + 2], 1.0)
                    v_sb[t] = vt

                # ---- pair-0 qk then the first v tile upfront (overlaps the
                # input DMAs; qk only needs x + wq/wk, which land first)
                for step in range(8):
                    emit_qk_part(0, step)
                emit_v(0)

                def filler(h, j):
                    # PE work emitted while ACT runs exp: v tiles (head 0),
                    # all of pair 1's q/k (head 1), then half a pair per head
                    # (accumulate on j=1..3, drain-copy on j=5: the copy sits
                    # well clear of both the next half's psum reuse and the
                    # next head's first scores)
                    if h == 0:
                        if j < NT - 1:
                            emit_v(j + 1)
                    elif h == 1:
                        emit_qk_part(1, j)
                    elif h <= 9:
                        fp = h // 2 + 1
                        if 1 <= j <= 3:
                            emit_qk_part(fp, (h % 2) * 4 + (j - 1))
                        elif j == 5:
                            emit_qk_part(fp, (h % 2) * 4 + 3)

                def emit_norm(h, ps_o):
                    # Immediate part: drain the psum accumulator fast (sums
                    # row -> SBUF on DVE, rows 0:64 on ACT, so the single ot
                    # slot frees in ~1us), run the (slow, ~6.5us, single-lane)
                    # DVE reciprocal from the SBUF copy, and bounce the result
                    # through DRAM (the only partition-broadcast path this
                    # toolchain supports).  The broadcast-read + multiply is
                    # deferred a full head (emit_norm_flush) so nothing on the
                    # PE queue ever waits on any of it.
                    p, r = h // 2, (h % 2) * DH
                    if r == 0:
                        u_sb = un_pool.tile([128, T], F32, tag="un", name=f"u{p}")
                        emit_v.u = u_sb
                    u_sb = emit_v.u
                    nc.scalar.copy(u_sb[r : r + DH, :], ps_o[0:DH, :])
                    rc = small_pool.tile(
                        [1, T], F32, tag=f"rc{h % 2}", name=f"rc{h}", bufs=1
                    )
                    if h == HEADS - 1:
                        # tail: recip = exp(-ln(d)) on ACT straight from PSUM.
                        # Ln and Exp share a table set, so no table switches,
                        # and the 6.5us DVE reciprocal leaves the tail chain.
                        lnd = small_pool.tile(
                            [1, T], F32, tag="lnd", name="lnd", bufs=1
                        )
                        nc.scalar.activation(
                            lnd[:],
                            ps_o[DH : DH + 1, :],
                            mybir.ActivationFunctionType.Ln,
                        )
                        nc.scalar.activation(
                            rc[:],
                            lnd[:],
                            mybir.ActivationFunctionType.Exp,
                            scale=-1.0,
                        )
                    else:
                        srow = small_pool.tile(
                            [1, T], F32, tag=f"sr{h % 2}", name=f"sr{h}", bufs=1
                        )
                        nc.vector.tensor_copy(srow[:], ps_o[DH : DH + 1, :])
                        nc.vector.reciprocal(rc[:], srow[:])
                    nc.sync.dma_start(rscr[p][h % 2 : h % 2 + 1, :], rc[0:1, :])
                    if r != 0:
                        emit_norm.pending = (p, u_sb)

                emit_norm.pending = None

                def emit_norm_flush():
                    if emit_norm.pending is None:
                        return
                    p, u_sb = emit_norm.pending
                    emit_norm.pending = None
                    bc_sb = bc_pool.tile([128, T], F32, tag="bc", name=f"bc{p}")
                    nc.gpsimd.dma_start(
                        bc_sb[0:DH, :], rscr[p][0:1, :].to_broadcast((DH, T))
                    )
                    nc.gpsimd.dma_start(
                        bc_sb[DH:128, :], rscr[p][1:2, :].to_broadcast((DH, T))
                    )
                    o_sb[p] = o_pool.tile([128, T], DT, tag="op", name=f"o{p}")
                    with nc.allow_low_precision(reason="bf16 o tile"):
                        nc.vector.tensor_mul(o_sb[p][:, :], u_sb[:, :], bc_sb[:])

                def emit_av(h, j, e_sb):
                    # AV accumulation lags its exp by one j-slot so the PE
                    # never waits on ACT
                    if j == 0:
                        ps_o = ot_pool.tile([DH + 1, T], F32, tag="ot", name=f"ot{h}")
                        emit_av.cur = ps_o
                    ps_o = emit_av.cur
                    v_sl = v_sb[j][:, h, 0 : DH + 1]
                    _mm(nc, ps_o[:, 0:512], v_sl, e_sb[:, 0:512],
                        start=(j == 0), stop=(j == NT - 1))
                    _mm(nc, ps_o[:, 512:1024], v_sl, e_sb[:, 512:1024],
                        start=(j == 0), stop=(j == NT - 1))
                    if j == NT - 1:
                        emit_norm(h, ps_o)

                pend = None
                for h in range(HEADS):
                    p, r = h // 2, (h % 2) * DH
                    for j in range(NT):
                        ps_s = sc_pool.tile([128, T], F32, tag="sc", name=f"sc{h}_{j}")
                        kt_sl = k_sb[p][r : r + DH, j * 128 : (j + 1) * 128]
                        qt_row = q_sb[p][r : r + DH, :]
                        _mm(nc, ps_s[:, 0:512], kt_sl, qt_row[:, 0:512], start=True, stop=True)
                        _mm(nc, ps_s[:, 512:1024], kt_sl, qt_row[:, 512:1024], start=True, stop=True)
                        e_sb = exp_pool.tile([128, T], EDT, tag="exp", name=f"e{h}_{j}")
                        nc.scalar.activation(e_sb[:], ps_s[:], mybir.ActivationFunctionType.Exp)
                        filler(h, j)
                        if pend is not None:
                            emit_av(*pend)
                        pend = (h, j, e_sb)
                        if h % 2 == 1 and j == 1:
                            # pair (h-3)//2's broadcast+multiply: a full head
                            # after its reciprocal was issued, so the 6.5us
                            # DVE reciprocal is long done by the time the PE
                            # reaches these matmuls
                            emit_norm_flush()
                emit_av(*pend)
                emit_norm_flush()

                # ---- output projection (double-buffered via sc pool), staged
                # in [128, 2*DIM] tiles -> 4 output DMAs.  Chunk 5 (the last
                # pair, whose normalization lands latest) is deferred one
                # t-tile so its reciprocal+multiply hide under the other
                # chunks' matmuls.
                def finish_proj(t, ps_out):
                    c = NCH - 1
                    lhsT = o_sb[c][:, t * 128 : (t + 1) * 128]
                    _mm(nc, ps_out[:, 0:512], lhsT, wo_sb[:, c * DIM : c * DIM + 512],
                        start=False, stop=True)
                    _mm(nc, ps_out[:, 512:768], lhsT, wo_sb[:, c * DIM + 512 : (c + 1) * DIM],
                        start=False, stop=True)
                    if t % 2 == 0:
                        o_t = ost_pool.tile([128, 2 * DIM], F32, tag="ost", name=f"os{t // 2}")
                        emit_v.ost = o_t
                    o_t = emit_v.ost
                    nc.scalar.copy(o_t[:, (t % 2) * DIM : (t % 2 + 1) * DIM], ps_out[:])
                    if t % 2 == 1:
                        g = t // 2
                        dst = out[g * 256 : (g + 1) * 256, :].rearrange(
                            "(a p) d -> p a d", a=2
                        )
                        src = o_t[:].rearrange("p (a d) -> p a d", a=2)
                        nc.sync.dma_start(dst, src)

                def po_tile(t):
                    # 3 concurrent accumulators: the two sc-pool slots plus
                    # the (now idle) mm slot
                    if t % 3 == 2:
                        return mm_pool.tile([128, DIM], F32, tag="mm", name=f"po{t}")
                    return sc_pool.tile([128, DIM], F32, tag="sc", name=f"po{t}")

                pending_proj = []
                for t in range(NT):
                    ps_out = po_tile(t)
                    for c in range(NCH - 1):
                        lhsT = o_sb[c][:, t * 128 : (t + 1) * 128]
                        _mm(nc, ps_out[:, 0:512], lhsT, wo_sb[:, c * DIM : c * DIM + 512],
                            start=(c == 0), stop=False)
                        _mm(nc, ps_out[:, 512:768], lhsT, wo_sb[:, c * DIM + 512 : (c + 1) * DIM],
                            start=(c == 0), stop=False)
                    pending_proj.append((t, ps_out))
                    if len(pending_proj) > 2:
                        finish_proj(*pending_proj.pop(0))
                for item in pending_proj:
                    finish_proj(*item)

            for _rep in range(reps):
                _one_rep()

    if split:
        _split_sp_waits(nc)
    return nc


_NC_CACHE = {}


def _get_nc():
    if "nc" not in _NC_CACHE:
        _NC_CACHE["nc"] = build_program()
    return _NC_CACHE["nc"]


def _pack_rows(W):
    """[768, N] -> [128, 6*N]: chunk c of 128 rows lands at cols [c*N,(c+1)*N)."""
    n = W.shape[1]
    return np.ascontiguousarray(
        W.reshape(NCH, 128, n).transpose(1, 0, 2).reshape(128, NCH * n)
    )


def prep_inputs(x, Wqkv, bqkv, Wout, bout):
    """Host-side prep: per-core packed transposed x, head-major W packs."""
    import ml_dtypes

    assert not np.any(bqkv), "nonzero bqkv not supported"
    B = x.shape[0]
    ddt = ml_dtypes.bfloat16 if DT == BF16 else np.float32
    # Wqkv column c maps to (d, k, h): c = d*36 + k*12 + h
    w = np.ascontiguousarray(
        Wqkv.reshape(DIM, DH, 3, HEADS).transpose(0, 2, 3, 1)
    )  # [dd, k, h, d]
    wq = w[:, 0].reshape(DIM, DIM)
    wk = w[:, 1].reshape(DIM, DIM)
    wv = w[:, 2].reshape(DIM, DIM)
    wqkv = np.concatenate(
        [_pack_rows(wq), _pack_rows(wk), _pack_rows(wv)], axis=1
    ).astype(ddt)
    wo_p = _pack_rows(np.asarray(Wout)).astype(ddt)
    in_maps = []
    for b in range(B):
        in_maps.append(
            {
                "xt": _pack_rows(np.ascontiguousarray(x[b].T)).astype(ddt),
                "wqkv": wqkv,
                "wo": wo_p,
            }
        )
    return in_maps


def kernel(x, Wqkv, bqkv, Wout, bout, trace=False, tmpdir=None):
    x = np.asarray(x, dtype=np.float32)
    Wqkv = np.asarray(Wqkv, dtype=np.float32)
    bqkv = np.asarray(bqkv, dtype=np.float32)
    Wout = np.asarray(Wout, dtype=np.float32)
    bout = np.asarray(bout, dtype=np.float32)
    B = x.shape[0]
    assert B == 8 and x.shape[1] == T and x.shape[2] == DIM

    nc = _get_nc()
    in_maps = prep_inputs(x, Wqkv, bqkv, Wout, bout)
    res = run_bass_kernel_spmd(
        nc, in_maps, list(range(B)), trace=trace, tmpdir=tmpdir
    )
    out = np.stack([res.results[b]["out"] for b in range(B)], axis=0)
    if np.any(bout):
        out = out + bout
    kernel.last_result = res
    return out

